# revision 11
# baseline (speedup 1.0000x reference)
"""Trainium2 Bass kernel for 2D single-level DWT (coif1, symmetric padding).

Input  x: (4, 64, 512, 512) fp32
Output  : (4, 256, 258, 258) fp32  -- per input channel: [cA, cH, cV, cD]

Math: with R_f the banded 258x512 operator of the 1D DWT along an axis
(6-tap filter, stride 2, symmetric boundary folds), the four outputs are
    cA = R_lo X R_lo^T,  cH = R_hi X R_lo^T,
    cV = R_lo X R_hi^T,  cD = R_hi X R_hi^T.

v4 design (fp16 data path, band-windowed matmuls, 32 images per core):
  pass 1 (contract rows r):   Yt_f[c, kh] = sum_r X[r, c] R_f[kh, r]
     stationary lhsT = X chunk [r:128, c:128]; moving rhs = R^T slice with
     the lo/hi filter pair interleaved along the stream dim (col 2*kh+f),
     so one matmul serves both filters per LDWEIGHTS.  R is banded:
     r-chunk q only reaches kh in [64q, 64q+66), so each matmul streams
     ~132 interleaved columns instead of 516.
  pass 2 (contract cols c):   O_s[kh, kw] = sum_c Yt_f[c, kh] R_g[kw, c]
     stationary lhsT = Yt chunk (stride-2 slice of the interleaved Yt);
     kh tiled [0,128), [128,256), plus a 2-row remainder whose lhsT is the
     4 contiguous tail columns of each Yt block.
  PSUM accumulation relies on per-element has_written bits: first matmul
  into a bank uses start=True (arms lazy-zero for the whole bank); later
  chain matmuls use start=False and may touch a mix of written
  (accumulate) and pending-zero (overwrite) columns.
  Chains are PAIRED into double-width PSUM tiles (A-pair spans 2 banks,
  B-pair shares 1 bank) so one engine copy drains two chains -- the
  scalar engine pays ~200ns fixed cost per instruction, so fewer, bigger
  drains matter.  The PE runs pass1(i+1) before pass2(i) so drains always
  trail a full chain-group behind the producer (no PSUM-recycle stalls).
  DMA: 2-image granularity, 128 partitions x >=2KiB contiguous per
  partition per transfer (spreads over all 16 SDMA engines).
"""

import os
import sys

for _p in ("/opt/trn_rl_repo", "/opt/pypackages"):
    if _p not in sys.path:
        sys.path.append(_p)

os.environ.setdefault("JAX_COMPILATION_CACHE_DIR", "/tmp/jax_comp_cache")
os.environ.setdefault("JAX_PERSISTENT_CACHE_MIN_COMPILE_TIME_SECS", "10")

import numpy as np

import concourse.bass as bass
import concourse.bacc as bacc
import concourse.mybir as mybir
from concourse.bass_utils import run_bass_kernel_spmd
from concourse.tile import TileContext

N_CORES = 8
H = W = 512
OUT = 258  # (512 + 6 - 1) // 2
IMGS = 32  # images per core (4*64/8)
GRP = 2    # images per DMA transfer
NG = IMGS // GRP
F16 = mybir.dt.float16
F32 = mybir.dt.float32

# pywt coif1 decomposition filters
DEC_LO = np.array([-0.01565572813546454, -0.0727326195128539, 0.38486484686420286,
                   0.8525720202122554, 0.3378976624578092, -0.0727326195128539])
DEC_HI = np.array([0.0727326195128539, 0.3378976624578092, -0.8525720202122554,
                   0.38486484686420286, 0.0727326195128539, -0.01565572813546454])
FLEN = 6
PAD = 4
LO_F = DEC_LO[::-1]
HI_F = DEC_HI[::-1]

# kh/kw window that r/c-chunk q contributes to (from the band structure)
WINS = [(0, 66), (64, 130), (128, 194), (192, 258)]
BSP = 194  # per-chain PSUM split: [0,194)x2 = 1552B (A), [194,258)x2 = 512B (B)

# If True, split matmuls so no instruction touches a mix of
# already-written and pending-zero PSUM bytes (needed only for CoreSim;
# hardware has per-element has_written bits).
INTERP_SAFE = False


def _build_R(filt: np.ndarray, n: int = W) -> np.ndarray:
    """Banded [258, 512] operator: out[k] = sum_j filt[j] * x[sym(2k + j - PAD)]."""
    out_len = (n + FLEN - 1) // 2

    def sym(i: int) -> int:
        while i < 0 or i >= n:
            if i < 0:
                i = -i - 1
            if i >= n:
                i = 2 * n - 1 - i
        return i

    R = np.zeros((out_len, n), dtype=np.float64)
    for k in range(out_len):
        for j in range(FLEN):
            R[k, sym(2 * k + j - PAD)] += filt[j]
    return R


def _check_windows(R: np.ndarray) -> None:
    for q in range(4):
        nz = np.nonzero((R[:, 128 * q:128 * (q + 1)] != 0).any(axis=1))[0]
        assert (int(nz.min()), int(nz.max()) + 1) == WINS[q], (q, nz.min(), nz.max())


def _build_weights() -> np.ndarray:
    """Interleaved: w[p, q*516 + 2k + f] = R_f[k, 128q + p], [128, 4*516] fp16."""
    Rs = [_build_R(LO_F), _build_R(HI_F)]
    _check_windows(Rs[0])
    _check_windows(Rs[1])
    w = np.zeros((128, 4 * 2 * OUT), dtype=np.float32)
    for q in range(4):
        blk = np.zeros((128, OUT, 2), dtype=np.float32)
        for f in range(2):
            blk[:, :, f] = Rs[f][:, 128 * q:128 * (q + 1)].T
        w[:, q * 2 * OUT:(q + 1) * 2 * OUT] = blk.reshape(128, 2 * OUT)
    return w.astype(np.float16)


_WEIGHTS = _build_weights()
_MODULE = None


def _build_module() -> bass.Bass:
    nc = bacc.Bacc("TRN2", target_bir_lowering=False, debug=False)
    x_in = nc.declare_dram_parameter("x", [NG, 128, GRP * 4 * W], F16,
                                     isOutput=False)
    w_in = nc.declare_dram_parameter("w", [128, 4 * 2 * OUT], F16, isOutput=False)
    # y[g, p, ((ig*2 + khc)*2 + f)*516 + 2*kw + gg] = O_{f+2gg}[128*khc + p, kw]
    y_main = nc.declare_dram_parameter("y", [NG, 128, GRP * 4 * 516], F16,
                                       isOutput=True)
    # yr[j*4 + f*2 + ig, g*516 + 2*kw + gg] = O_{f+2gg}[256 + j, kw]
    y_rem = nc.declare_dram_parameter("yr", [8, NG * 516], F16,
                                     isOutput=True)

    with TileContext(nc) as tc:
        with (
            tc.tile_pool(name="wpool", bufs=1) as wpool,
            tc.tile_pool(name="xpool", bufs=3) as xpool,
            tc.tile_pool(name="ypool", bufs=2) as ypool,
            tc.tile_pool(name="spool", bufs=2) as spool,
            tc.tile_pool(name="rpool", bufs=1) as rpool,
            tc.tile_pool(name="psum", bufs=2, space="PSUM") as pspool,
        ):
            Wt = wpool.tile([128, 4 * 2 * OUT], F16)
            Crem = rpool.tile([8, NG * 516], F16)

            def load_x(g, split=False):
                X = xpool.tile([128, GRP * 4 * W], F16, tag="X", name=f"X_{g}")
                if split:
                    # engage many SDMA engines for the startup-critical load
                    for k in range(8):
                        ring = nc.sync if k % 2 == 0 else nc.gpsimd
                        ring.dma_start(out=X[:, k * W:(k + 1) * W],
                                       in_=x_in[g, :, k * W:(k + 1) * W])
                else:
                    nc.sync.dma_start(out=X[:, 0:4 * W],
                                      in_=x_in[g, :, 0:4 * W])
                    nc.sync.dma_start(out=X[:, 4 * W:8 * W],
                                      in_=x_in[g, :, 4 * W:8 * W])
                return X

            nc.sync.dma_start(out=Wt[:, 0:1032], in_=w_in[:, 0:1032])
            nc.gpsimd.dma_start(out=Wt[:, 1032:2064], in_=w_in[:, 1032:2064])
            X0 = load_x(0, split=True)
            Wr = Wt[:]

            # Tiny PE op consuming the weight DMA so later matmuls depend
            # on it via PE program order.
            warm = pspool.tile([1, 256], F32, tag="pBB", bufs=2)
            nc.tensor.matmul(warm[:, 0:1], lhsT=Wr[:, 0:1], rhs=Wr[:, 0:1],
                             start=True, stop=True)

            def copy(dst, src, eng):
                if eng == "s":
                    nc.scalar.copy(out=dst, in_=src)
                else:
                    nc.vector.tensor_copy(out=dst, in_=src)

            def chain(lhsT_fn, A, B, ha, hb):
                """One banded, filter-interleaved accumulation chain into
                half `ha` of A-pair tile A (512-elem halves = bank-aligned)
                and half `hb` of B-pair tile B (128-elem halves)."""
                if INTERP_SAFE:
                    segs = [(0, 0, 66, 0, True, False),
                            (1, 64, 66, 0, False, False),
                            (1, 66, 130, 0, False, False),
                            (2, 128, 130, 0, False, False),
                            (2, 130, 194, 0, False, False),
                            (3, 192, 194, 0, False, True),
                            (3, 194, 258, 1, True, True)]
                else:
                    segs = [(0, 0, 66, 0, True, False),
                            (1, 64, 130, 0, False, False),
                            (2, 128, 194, 0, False, False),
                            (3, 192, 194, 0, False, True),
                            (3, 194, 258, 1, True, True)]
                for q, lo, hi, t, st, sp in segs:
                    if t == 0:
                        out = A[:, ha * 512 + 2 * lo:ha * 512 + 2 * hi]
                    else:
                        out = B[:, hb * 128 + 2 * (lo - BSP):
                                hb * 128 + 2 * (hi - BSP)]
                    rhs = Wr[:, q * 2 * OUT + 2 * lo:q * 2 * OUT + 2 * hi]
                    nc.tensor.matmul(out, lhsT=lhsT_fn(q), rhs=rhs,
                                     start=st, stop=sp)

            def pair_views(A, B):
                Ah = A[:].rearrange("p (h k) -> p h k", h=2)[:, :, 0:2 * BSP]
                Bh = B[:].rearrange("p (h k) -> p h k", h=2)
                return Ah, Bh

            def pass1(Xv, Yt, ig):
                """4 paired chains into the group Yt tile:
                Yt[p, cc*1032 + (2*kh + f)*2 + ig]."""
                Ytv = Yt[:].rearrange("p (cc k i) -> p cc k i", cc=4, i=GRP)
                for cp in range(2):  # cc pairs (0,1), (2,3)
                    A = pspool.tile([128, 1024], F32, tag="pAA", bufs=3)
                    B = pspool.tile([128, 256], F32, tag="pBB", bufs=2)
                    for h in range(2):
                        cc = cp * 2 + h
                        chain(lambda q: Xv[:, ig, q, cc * 128:(cc + 1) * 128],
                              A[:], B[:], h, h)
                    Ah, Bh = pair_views(A, B)
                    copy(Ytv[:, 2 * cp:2 * cp + 2, 0:2 * BSP, ig], Ah,
                         "s" if cp == 0 else "v")
                    copy(Ytv[:, 2 * cp:2 * cp + 2, 2 * BSP:516, ig], Bh,
                         "v" if cp == 0 else "s")

            def pass2(Yt, STG, ig, i):
                Ytr = Yt[:]
                Ytv4 = Ytr.rearrange("p (cc k f i) -> p cc k f i",
                                     cc=4, f=2, i=GRP)
                Sv = STG[:].rearrange("p (blk k) -> p blk k", k=516)
                for khc in range(2):  # pair over f
                    A = pspool.tile([128, 1024], F32, tag="pAA", bufs=3)
                    B = pspool.tile([128, 256], F32, tag="pBB", bufs=2)
                    for f in range(2):
                        chain(lambda q: Ytv4[:, q,
                                             128 * khc:128 * (khc + 1), f, ig],
                              A[:], B[:], f, f)
                    Ah, Bh = pair_views(A, B)
                    base = (ig * 2 + khc) * 2
                    copy(Sv[:, base:base + 2, 0:2 * BSP], Ah,
                         "s" if khc == 0 else "v")
                    copy(Sv[:, base:base + 2, 2 * BSP:516], Bh,
                         "v" if khc == 0 else "s")

            def rem(Yt, g):
                # remainder rows kh in {256,257} for the whole group: lhsT =
                # 8 contiguous tail cols of each Yt block (order (j, f, ig));
                # psum rows j*4 + f*2 + ig.  One pAA tile: A part in bank 0,
                # B part at the start of bank 1.
                Ytr = Yt[:]
                Rt = pspool.tile([8, 1024], F32, tag="pAA", bufs=3)
                chain(lambda q: Ytr[:, q * 1032 + 1024:(q + 1) * 1032],
                      Rt[:], Rt[:], 0, 4)
                copy(Crem[:, g * 516:g * 516 + 2 * BSP], Rt[:, 0:2 * BSP], "v")
                copy(Crem[:, g * 516 + 2 * BSP:(g + 1) * 516],
                     Rt[:, 512:640], "s")

            # software pipeline: PE runs pass1(i+1) before pass2(i)
            Xg = {0: X0, 1: load_x(1)}
            Xv = {g: Xg[g][:].rearrange("p (i q c) -> p i q c", i=GRP, q=4)
                  for g in (0, 1)}
            Ytg = {0: ypool.tile([128, 4 * 516 * GRP], F16, tag="Yt",
                     name="Yt_0")}
            pass1(Xv[0], Ytg[0], 0)
            STG = None
            for i in range(IMGS):
                g, ig = divmod(i, GRP)
                if ig == 0:
                    if g + 2 < NG:
                        Xg[g + 2] = load_x(g + 2)
                        Xv[g + 2] = Xg[g + 2][:].rearrange(
                            "p (i q c) -> p i q c", i=GRP, q=4)
                    STG = spool.tile([128, GRP * 4 * 516], F16, tag="STG")
                if i + 1 < IMGS:
                    g1, ig1 = divmod(i + 1, GRP)
                    if g1 not in Ytg:
                        Ytg[g1] = ypool.tile([128, 4 * 516 * GRP], F16,
                                             tag="Yt", name=f"Yt_{g1}")
                    pass1(Xv[g1], Ytg[g1], ig1)
                pass2(Ytg[g], STG, ig, i)
                if ig == 0:
                    nc.gpsimd.dma_start(out=y_main[g, :, 0:4 * 516],
                                        in_=STG[:, 0:4 * 516])
                if ig == GRP - 1:
                    rem(Ytg[g], g)
                    del Ytg[g]
                    if g == NG - 1:
                        nc.gpsimd.dma_start(out=y_main[g, :, 4 * 516:6 * 516],
                                            in_=STG[:, 4 * 516:6 * 516])
                        nc.sync.dma_start(out=y_main[g, :, 6 * 516:8 * 516],
                                          in_=STG[:, 6 * 516:8 * 516])
                    else:
                        nc.gpsimd.dma_start(out=y_main[g, :, 4 * 516:8 * 516],
                                            in_=STG[:, 4 * 516:8 * 516])
                if i % 8 == 7:
                    c = i // 8
                    nc.gpsimd.dma_start(
                        out=y_rem[:, c * 4 * 516:(c + 1) * 4 * 516],
                        in_=Crem[:, c * 4 * 516:(c + 1) * 4 * 516])
    nc.finalize()
    return nc


def _get_module() -> bass.Bass:
    global _MODULE
    if _MODULE is None:
        _MODULE = _build_module()
    return _MODULE


def make_in_maps(x: np.ndarray) -> list[dict]:
    x = np.asarray(x, dtype=np.float32)
    B, C, Hx, Wx = x.shape
    assert (Hx, Wx) == (H, W) and B * C == N_CORES * IMGS
    imgs = x.reshape(B * C, H, W)
    maps = []
    for k in range(N_CORES):
        # X[g][p, ig*2048 + q*512 + c] = x[g*GRP+ig, 128q + p, c]
        xc = imgs[k * IMGS:(k + 1) * IMGS].reshape(NG, GRP, 4, 128, W)
        xc = np.ascontiguousarray(xc.transpose(0, 3, 1, 2, 4))
        maps.append({"x": xc.reshape(NG, 128, GRP * 4 * W).astype(np.float16),
                     "w": _WEIGHTS})
    return maps


def kernel(**inputs) -> np.ndarray:
    x = np.asarray(inputs["x"], dtype=np.float32)
    B, C, Hx, Wx = x.shape

    nc = _get_module()
    in_maps = make_in_maps(x)
    res = run_bass_kernel_spmd(nc, in_maps, list(range(N_CORES))).results

    full = np.empty((N_CORES * IMGS, 4, OUT, OUT), dtype=np.float32)
    for k in range(N_CORES):
        # [g, p, ig, khc, f, kw, gg]
        ym = res[k]["y"].reshape(NG, 128, GRP, 2, 2, OUT, 2)
        yr = res[k]["yr"].reshape(8, NG, OUT, 2)  # [j*4+f*2+ig, g, kw, gg]
        dst = full[k * IMGS:(k + 1) * IMGS]
        # dst[g*GRP+ig, f+2gg, khc*128+p, kw] = ym[g, p, ig, khc, f, kw, gg]
        t = ym.transpose(0, 2, 4, 6, 3, 1, 5).reshape(IMGS, 4, 256, OUT)
        # t's dim-1 is f*2+gg; reorder to s = f+2gg -> fg indices [0,2,1,3]
        dst[:, :, :256, :] = t[:, [0, 2, 1, 3]]
        for f in range(2):
            for j in range(2):
                for g in range(2):
                    for ig in range(GRP):
                        dst[ig::GRP, f + 2 * g, 256 + j, :] = \
                            yr[j * 4 + f * 2 + ig, :, :, g]

    return np.ascontiguousarray(full.reshape(B, 4 * C, OUT, OUT))



# revision 12
# speedup vs baseline: 1.0097x; 1.0097x over previous
"""Trainium2 Bass kernel for 2D single-level DWT (coif1, symmetric padding).

Input  x: (4, 64, 512, 512) fp32
Output  : (4, 256, 258, 258) fp32  -- per input channel: [cA, cH, cV, cD]

Math: with R_f the banded 258x512 operator of the 1D DWT along an axis
(6-tap filter, stride 2, symmetric boundary folds), the four outputs are
    cA = R_lo X R_lo^T,  cH = R_hi X R_lo^T,
    cV = R_lo X R_hi^T,  cD = R_hi X R_hi^T.

v4 design (fp16 data path, band-windowed matmuls, 32 images per core):
  pass 1 (contract rows r):   Yt_f[c, kh] = sum_r X[r, c] R_f[kh, r]
     stationary lhsT = X chunk [r:128, c:128]; moving rhs = R^T slice with
     the lo/hi filter pair interleaved along the stream dim (col 2*kh+f),
     so one matmul serves both filters per LDWEIGHTS.  R is banded:
     r-chunk q only reaches kh in [64q, 64q+66), so each matmul streams
     ~132 interleaved columns instead of 516.
  pass 2 (contract cols c):   O_s[kh, kw] = sum_c Yt_f[c, kh] R_g[kw, c]
     stationary lhsT = Yt chunk (stride-2 slice of the interleaved Yt);
     kh tiled [0,128), [128,256), plus a 2-row remainder whose lhsT is the
     4 contiguous tail columns of each Yt block.
  PSUM accumulation relies on per-element has_written bits: first matmul
  into a bank uses start=True (arms lazy-zero for the whole bank); later
  chain matmuls use start=False and may touch a mix of written
  (accumulate) and pending-zero (overwrite) columns.
  Chains are PAIRED into double-width PSUM tiles (A-pair spans 2 banks,
  B-pair shares 1 bank) so one engine copy drains two chains -- the
  scalar engine pays ~200ns fixed cost per instruction, so fewer, bigger
  drains matter.  The PE runs pass1(i+1) before pass2(i) so drains always
  trail a full chain-group behind the producer (no PSUM-recycle stalls).
  DMA: 2-image granularity, 128 partitions x >=2KiB contiguous per
  partition per transfer (spreads over all 16 SDMA engines).
"""

import os
import sys

for _p in ("/opt/trn_rl_repo", "/opt/pypackages"):
    if _p not in sys.path:
        sys.path.append(_p)

os.environ.setdefault("JAX_COMPILATION_CACHE_DIR", "/tmp/jax_comp_cache")
os.environ.setdefault("JAX_PERSISTENT_CACHE_MIN_COMPILE_TIME_SECS", "10")

import numpy as np

import concourse.bass as bass
import concourse.bacc as bacc
import concourse.mybir as mybir
from concourse.bass_utils import run_bass_kernel_spmd
from concourse.tile import TileContext

N_CORES = 8
H = W = 512
OUT = 258  # (512 + 6 - 1) // 2
IMGS = 32  # images per core (4*64/8)
GRP = 2    # images per DMA transfer
NG = IMGS // GRP
F16 = mybir.dt.float16
F32 = mybir.dt.float32

# pywt coif1 decomposition filters
DEC_LO = np.array([-0.01565572813546454, -0.0727326195128539, 0.38486484686420286,
                   0.8525720202122554, 0.3378976624578092, -0.0727326195128539])
DEC_HI = np.array([0.0727326195128539, 0.3378976624578092, -0.8525720202122554,
                   0.38486484686420286, 0.0727326195128539, -0.01565572813546454])
FLEN = 6
PAD = 4
LO_F = DEC_LO[::-1]
HI_F = DEC_HI[::-1]

# kh/kw window that r/c-chunk q contributes to (from the band structure)
WINS = [(0, 66), (64, 130), (128, 194), (192, 258)]
BSP = 194  # per-chain PSUM split: [0,194)x2 = 1552B (A), [194,258)x2 = 512B (B)

# If True, split matmuls so no instruction touches a mix of
# already-written and pending-zero PSUM bytes (needed only for CoreSim;
# hardware has per-element has_written bits).
INTERP_SAFE = False


def _build_R(filt: np.ndarray, n: int = W) -> np.ndarray:
    """Banded [258, 512] operator: out[k] = sum_j filt[j] * x[sym(2k + j - PAD)]."""
    out_len = (n + FLEN - 1) // 2

    def sym(i: int) -> int:
        while i < 0 or i >= n:
            if i < 0:
                i = -i - 1
            if i >= n:
                i = 2 * n - 1 - i
        return i

    R = np.zeros((out_len, n), dtype=np.float64)
    for k in range(out_len):
        for j in range(FLEN):
            R[k, sym(2 * k + j - PAD)] += filt[j]
    return R


def _check_windows(R: np.ndarray) -> None:
    for q in range(4):
        nz = np.nonzero((R[:, 128 * q:128 * (q + 1)] != 0).any(axis=1))[0]
        assert (int(nz.min()), int(nz.max()) + 1) == WINS[q], (q, nz.min(), nz.max())


def _build_weights() -> np.ndarray:
    """Interleaved: w[p, q*516 + 2k + f] = R_f[k, 128q + p], [128, 4*516] fp16."""
    Rs = [_build_R(LO_F), _build_R(HI_F)]
    _check_windows(Rs[0])
    _check_windows(Rs[1])
    w = np.zeros((128, 4 * 2 * OUT), dtype=np.float32)
    for q in range(4):
        blk = np.zeros((128, OUT, 2), dtype=np.float32)
        for f in range(2):
            blk[:, :, f] = Rs[f][:, 128 * q:128 * (q + 1)].T
        w[:, q * 2 * OUT:(q + 1) * 2 * OUT] = blk.reshape(128, 2 * OUT)
    return w.astype(np.float16)


_WEIGHTS = _build_weights()
_MODULE = None


def _build_module() -> bass.Bass:
    nc = bacc.Bacc("TRN2", target_bir_lowering=False, debug=False)
    x_in = nc.declare_dram_parameter("x", [NG, 128, GRP * 4 * W], F16,
                                     isOutput=False)
    w_in = nc.declare_dram_parameter("w", [128, 4 * 2 * OUT], F16, isOutput=False)
    # y[g, p, ((ig*2 + khc)*2 + f)*516 + 2*kw + gg] = O_{f+2gg}[128*khc + p, kw]
    y_main = nc.declare_dram_parameter("y", [NG, 128, GRP * 4 * 516], F16,
                                       isOutput=True)
    # yr[j*4 + f*2 + ig, g*516 + 2*kw + gg] = O_{f+2gg}[256 + j, kw]
    y_rem = nc.declare_dram_parameter("yr", [8, NG * 516], F16,
                                     isOutput=True)

    with TileContext(nc) as tc:
        with (
            tc.tile_pool(name="wpool", bufs=1) as wpool,
            tc.tile_pool(name="xpool", bufs=3) as xpool,
            tc.tile_pool(name="ypool", bufs=2) as ypool,
            tc.tile_pool(name="spool", bufs=2) as spool,
            tc.tile_pool(name="rpool", bufs=1) as rpool,
            tc.tile_pool(name="psum", bufs=2, space="PSUM") as pspool,
        ):
            Wt = wpool.tile([128, 4 * 2 * OUT], F16)
            Crem = rpool.tile([8, NG * 516], F16)

            def load_x(g, split=False):
                X = xpool.tile([128, GRP * 4 * W], F16, tag="X", name=f"X_{g}")
                ring2 = nc.gpsimd if split else nc.sync
                nc.sync.dma_start(out=X[:, 0:4 * W], in_=x_in[g, :, 0:4 * W])
                ring2.dma_start(out=X[:, 4 * W:8 * W],
                                in_=x_in[g, :, 4 * W:8 * W])
                return X

            nc.sync.dma_start(out=Wt[:], in_=w_in[:])
            X0 = load_x(0, split=True)
            Wr = Wt[:]

            # Tiny PE op consuming the weight DMA so later matmuls depend
            # on it via PE program order.
            warm = pspool.tile([1, 256], F32, tag="pBB", bufs=2)
            nc.tensor.matmul(warm[:, 0:1], lhsT=Wr[:, 0:1], rhs=Wr[:, 0:1],
                             start=True, stop=True)

            def copy(dst, src, eng):
                if eng == "s":
                    nc.scalar.copy(out=dst, in_=src)
                else:
                    nc.vector.tensor_copy(out=dst, in_=src)

            def chain(lhsT_fn, A, B, ha, hb):
                """One banded, filter-interleaved accumulation chain into
                half `ha` of A-pair tile A (512-elem halves = bank-aligned)
                and half `hb` of B-pair tile B (128-elem halves)."""
                if INTERP_SAFE:
                    segs = [(0, 0, 66, 0, True, False),
                            (1, 64, 66, 0, False, False),
                            (1, 66, 130, 0, False, False),
                            (2, 128, 130, 0, False, False),
                            (2, 130, 194, 0, False, False),
                            (3, 192, 194, 0, False, True),
                            (3, 194, 258, 1, True, True)]
                else:
                    segs = [(0, 0, 66, 0, True, False),
                            (1, 64, 130, 0, False, False),
                            (2, 128, 194, 0, False, False),
                            (3, 192, 194, 0, False, True),
                            (3, 194, 258, 1, True, True)]
                for q, lo, hi, t, st, sp in segs:
                    if t == 0:
                        out = A[:, ha * 512 + 2 * lo:ha * 512 + 2 * hi]
                    else:
                        out = B[:, hb * 128 + 2 * (lo - BSP):
                                hb * 128 + 2 * (hi - BSP)]
                    rhs = Wr[:, q * 2 * OUT + 2 * lo:q * 2 * OUT + 2 * hi]
                    nc.tensor.matmul(out, lhsT=lhsT_fn(q), rhs=rhs,
                                     start=st, stop=sp)

            def pair_views(A, B):
                Ah = A[:].rearrange("p (h k) -> p h k", h=2)[:, :, 0:2 * BSP]
                Bh = B[:].rearrange("p (h k) -> p h k", h=2)
                return Ah, Bh

            def pass1(Xv, Yt, ig):
                """4 paired chains into the group Yt tile:
                Yt[p, cc*1032 + (2*kh + f)*2 + ig]."""
                Ytv = Yt[:].rearrange("p (cc k i) -> p cc k i", cc=4, i=GRP)
                for cp in range(2):  # cc pairs (0,1), (2,3)
                    A = pspool.tile([128, 1024], F32, tag="pAA", bufs=3)
                    B = pspool.tile([128, 256], F32, tag="pBB", bufs=2)
                    for h in range(2):
                        cc = cp * 2 + h
                        chain(lambda q: Xv[:, ig, q, cc * 128:(cc + 1) * 128],
                              A[:], B[:], h, h)
                    Ah, Bh = pair_views(A, B)
                    copy(Ytv[:, 2 * cp:2 * cp + 2, 0:2 * BSP, ig], Ah,
                         "s" if cp == 0 else "v")
                    copy(Ytv[:, 2 * cp:2 * cp + 2, 2 * BSP:516, ig], Bh,
                         "v" if cp == 0 else "s")

            def pass2(Yt, STG, ig, i):
                Ytr = Yt[:]
                Ytv4 = Ytr.rearrange("p (cc k f i) -> p cc k f i",
                                     cc=4, f=2, i=GRP)
                Sv = STG[:].rearrange("p (blk k) -> p blk k", k=516)
                for khc in range(2):  # pair over f
                    A = pspool.tile([128, 1024], F32, tag="pAA", bufs=3)
                    B = pspool.tile([128, 256], F32, tag="pBB", bufs=2)
                    for f in range(2):
                        chain(lambda q: Ytv4[:, q,
                                             128 * khc:128 * (khc + 1), f, ig],
                              A[:], B[:], f, f)
                    Ah, Bh = pair_views(A, B)
                    base = (ig * 2 + khc) * 2
                    copy(Sv[:, base:base + 2, 0:2 * BSP], Ah,
                         "s" if khc == 0 else "v")
                    copy(Sv[:, base:base + 2, 2 * BSP:516], Bh,
                         "v" if khc == 0 else "s")

            def rem(Yt, g):
                # remainder rows kh in {256,257} for the whole group: lhsT =
                # 8 contiguous tail cols of each Yt block (order (j, f, ig));
                # psum rows j*4 + f*2 + ig.  One pAA tile: A part in bank 0,
                # B part at the start of bank 1.
                Ytr = Yt[:]
                Rt = pspool.tile([8, 1024], F32, tag="pAA", bufs=3)
                chain(lambda q: Ytr[:, q * 1032 + 1024:(q + 1) * 1032],
                      Rt[:], Rt[:], 0, 4)
                copy(Crem[:, g * 516:g * 516 + 2 * BSP], Rt[:, 0:2 * BSP], "v")
                copy(Crem[:, g * 516 + 2 * BSP:(g + 1) * 516],
                     Rt[:, 512:640], "s")

            # software pipeline: PE runs pass1(i+1) before pass2(i)
            Xg = {0: X0, 1: load_x(1)}
            Xv = {g: Xg[g][:].rearrange("p (i q c) -> p i q c", i=GRP, q=4)
                  for g in (0, 1)}
            Ytg = {0: ypool.tile([128, 4 * 516 * GRP], F16, tag="Yt",
                     name="Yt_0")}
            pass1(Xv[0], Ytg[0], 0)
            STG = None
            for i in range(IMGS):
                g, ig = divmod(i, GRP)
                if ig == 0:
                    if g + 2 < NG:
                        Xg[g + 2] = load_x(g + 2)
                        Xv[g + 2] = Xg[g + 2][:].rearrange(
                            "p (i q c) -> p i q c", i=GRP, q=4)
                    STG = spool.tile([128, GRP * 4 * 516], F16, tag="STG")
                if i + 1 < IMGS:
                    g1, ig1 = divmod(i + 1, GRP)
                    if g1 not in Ytg:
                        Ytg[g1] = ypool.tile([128, 4 * 516 * GRP], F16,
                                             tag="Yt", name=f"Yt_{g1}")
                    pass1(Xv[g1], Ytg[g1], ig1)
                pass2(Ytg[g], STG, ig, i)
                if ig == 0:
                    nc.gpsimd.dma_start(out=y_main[g, :, 0:4 * 516],
                                        in_=STG[:, 0:4 * 516])
                if ig == GRP - 1:
                    rem(Ytg[g], g)
                    del Ytg[g]
                    if g == NG - 1:
                        nc.gpsimd.dma_start(out=y_main[g, :, 4 * 516:6 * 516],
                                            in_=STG[:, 4 * 516:6 * 516])
                        nc.sync.dma_start(out=y_main[g, :, 6 * 516:8 * 516],
                                          in_=STG[:, 6 * 516:8 * 516])
                    else:
                        nc.gpsimd.dma_start(out=y_main[g, :, 4 * 516:8 * 516],
                                            in_=STG[:, 4 * 516:8 * 516])
                if i % 8 == 7:
                    c = i // 8
                    nc.gpsimd.dma_start(
                        out=y_rem[:, c * 4 * 516:(c + 1) * 4 * 516],
                        in_=Crem[:, c * 4 * 516:(c + 1) * 4 * 516])
    nc.finalize()
    return nc


def _get_module() -> bass.Bass:
    global _MODULE
    if _MODULE is None:
        _MODULE = _build_module()
    return _MODULE


def make_in_maps(x: np.ndarray) -> list[dict]:
    x = np.asarray(x, dtype=np.float32)
    B, C, Hx, Wx = x.shape
    assert (Hx, Wx) == (H, W) and B * C == N_CORES * IMGS
    imgs = x.reshape(B * C, H, W)
    maps = []
    for k in range(N_CORES):
        # X[g][p, ig*2048 + q*512 + c] = x[g*GRP+ig, 128q + p, c]
        xc = imgs[k * IMGS:(k + 1) * IMGS].reshape(NG, GRP, 4, 128, W)
        xc = np.ascontiguousarray(xc.transpose(0, 3, 1, 2, 4))
        maps.append({"x": xc.reshape(NG, 128, GRP * 4 * W).astype(np.float16),
                     "w": _WEIGHTS})
    return maps


def kernel(**inputs) -> np.ndarray:
    x = np.asarray(inputs["x"], dtype=np.float32)
    B, C, Hx, Wx = x.shape

    nc = _get_module()
    in_maps = make_in_maps(x)
    res = run_bass_kernel_spmd(nc, in_maps, list(range(N_CORES))).results

    full = np.empty((N_CORES * IMGS, 4, OUT, OUT), dtype=np.float32)
    for k in range(N_CORES):
        # [g, p, ig, khc, f, kw, gg]
        ym = res[k]["y"].reshape(NG, 128, GRP, 2, 2, OUT, 2)
        yr = res[k]["yr"].reshape(8, NG, OUT, 2)  # [j*4+f*2+ig, g, kw, gg]
        dst = full[k * IMGS:(k + 1) * IMGS]
        # dst[g*GRP+ig, f+2gg, khc*128+p, kw] = ym[g, p, ig, khc, f, kw, gg]
        t = ym.transpose(0, 2, 4, 6, 3, 1, 5).reshape(IMGS, 4, 256, OUT)
        # t's dim-1 is f*2+gg; reorder to s = f+2gg -> fg indices [0,2,1,3]
        dst[:, :, :256, :] = t[:, [0, 2, 1, 3]]
        for f in range(2):
            for j in range(2):
                for g in range(2):
                    for ig in range(GRP):
                        dst[ig::GRP, f + 2 * g, 256 + j, :] = \
                            yr[j * 4 + f * 2 + ig, :, :, g]

    return np.ascontiguousarray(full.reshape(B, 4 * C, OUT, OUT))

